# revision 1
# baseline (speedup 1.0000x reference)
"""Handshaking kernel ('cat' type) for Trainium2, 8 NeuronCores.

Math: for each upper-triangular pair (i, j>=i):
    out[b, p(i,j), :] = tanh(W1 @ h_i + W2 @ h_j + bias),  W = [W1 | W2]

Decomposition: per-token projections A = seq @ W1^T + bias and C = seq @ W2^T
(small matmuls), then each output row is A[i] + C[j] followed by tanh — the
pair expansion is pure data movement, done on the PE via 0/1 "band" matmuls
(slices of one [128, 384] shifted-identity constant select shifted row ranges
of C/A into PSUM), DVE adds the partition-aligned A terms in place, ACT does
tanh while evicting PSUM->SBUF, and an indirect DMA scatters each tile's rows
to their pair-major positions in DRAM.

Sharding: 8 cores = 4 batches x 2 halves of the hidden dim (H=768 -> 384 per
core). All cores run the identical program (SPMD); per-core behavior differs
only through input tensors.
"""

import sys
import numpy as np

for _p in ("/opt/trn_rl_repo", "/root/.axon_site/_ro/trn_rl_repo"):
    if _p not in sys.path:
        sys.path.insert(0, _p)

B, L, H = 4, 256, 768
HH = H // 2          # per-core hidden slice
NPAIR = L * (L + 1) // 2   # 32896
BIG = 1 << 30        # scatter index for gap rows (dropped via bounds_check)
NPE_A = 0            # how many T1 tiles do the A-add on PE instead of DVE

# offset of pair (i, i) in the flattened pair dim; pair (i, j) -> OFF[i] + j - i
OFF = np.array([i * L - (i * (i - 1)) // 2 for i in range(L)], dtype=np.int64)


def _qa(e):  # 32-blocks used by a T2a segment of diag e (length 128 - e)
    return -(-(128 - e) // 32)


def build_schedule():
    """Tile schedule, identical for every core.

    Returns list of tiles; each tile is a dict:
      mms: list of (s, mb, mlen, rhs, start, stop) band matmuls:
           psum[mb:mb+mlen, :] (+)= PADI[:, s:s+mlen].T @ rhs
      tts: list of (mb, n, a): DVE in-place psum[mb:mb+n, :] += shifted-a
      idx: np.ndarray [128] int64 scatter row indices (BIG for gaps)
    """
    tiles = []

    # T1_d: rows (i, i+d), i in [0, 128), for d in [0, 128)
    for d in range(128):
        mms = [[128 + d, 0, 128, "C0", True, False]]
        if d > 0:
            mms.append([d, 0, 128, "C1", False, False])
        tts = []
        if d < NPE_A:
            mms.append([128, 0, 128, "A0", False, False])
        else:
            tts.append((0, 128, "A0"))
        mms[-1][5] = True
        idx = OFF[np.arange(128)] + d
        tiles.append(dict(mms=mms, tts=tts, idx=idx))

    # T2 segments. Each covers one run of a diagonal with band matmuls:
    #  kind 'a': diag e, rows i = 128+m', len 128-e, C-band s = 128+e, A = A1
    #  kind 'b': diag 256-e2, rows i = m', len e2, C-band s = 256-e2, A = A0
    # Every length 1..128 occurs exactly twice (once per kind).
    segs = [dict(kind="a", e=e, ln=128 - e) for e in range(128)]
    segs += [dict(kind="b", e=e2, ln=e2) for e2 in range(1, 129)]

    def seg_bands(sg):
        if sg["kind"] == "a":
            return 128 + sg["e"], "C1", "A1"
        return 256 - sg["e"], "C1", "A0"

    def seg_idx(sg):
        if sg["kind"] == "a":
            return OFF[128 + np.arange(sg["ln"])] + sg["e"]
        return OFF[np.arange(sg["ln"])] + (256 - sg["e"])

    def t2bin(base_sg, tail_sg, tail_mb):
        s_c, _, a_src = seg_bands(base_sg)
        # base-0 C band, extended to full M (auto-clips to zeros beyond ln)
        mms = [[s_c, 0, 128, "C1", True, False]]
        tts = [(0, base_sg["ln"], a_src)]
        idx = np.full(128, BIG, dtype=np.int64)
        idx[: base_sg["ln"]] = seg_idx(base_sg)
        if tail_sg is not None:
            ln = tail_sg["ln"]
            s_c2, _, a_src2 = seg_bands(tail_sg)
            mms.append([s_c2, tail_mb, ln, "C1", False, False])
            tts.append((tail_mb, ln, f"{a_src2}@{tail_mb}"))
            idx[tail_mb : tail_mb + ln] = seg_idx(tail_sg)
        mms[-1][5] = True
        return dict(mms=mms, tts=tts, idx=idx)

    by_len = {}
    for sg in segs:
        by_len.setdefault(sg["ln"], []).append(sg)

    giants = [sg for ln in range(97, 129) for sg in by_len[ln]]
    bigs = [sg for ln in range(65, 97) for sg in by_len[ln]]
    smalls = [sg for ln in range(1, 33) for sg in by_len[ln]]
    meds = [sg for ln in range(33, 65) for sg in by_len[ln]]
    assert len(giants) == 64 and len(bigs) == 64
    assert len(smalls) == 64 and len(meds) == 64

    for sg in giants:
        tiles.append(t2bin(sg, None, 0))
    for sg, sm in zip(bigs, smalls):
        tiles.append(t2bin(sg, sm, 96))       # small (<=32) at base 96
    for sg, sg2 in zip(meds[:32], meds[32:]):
        tiles.append(t2bin(sg, sg2, 64))      # medium (<=64) at base 64

    # coverage check: every pair row exactly once
    allidx = np.concatenate([t["idx"] for t in tiles])
    used = allidx[allidx < BIG]
    assert used.size == NPAIR and np.unique(used).size == NPAIR
    return tiles


TILES = build_schedule()
NT = len(TILES)  # 288


def _schedule_selfcheck():
    """Verify the band schedule reproduces A[i] + C[j] with random A, C."""
    rng = np.random.RandomState(0)
    Cc = rng.randn(L, 8).astype(np.float64)
    Aa = rng.randn(L, 8).astype(np.float64)
    srcs = {"C0": Cc[:128], "C1": Cc[128:], "A0": Aa[:128], "A1": Aa[128:]}
    padi = np.zeros((128, 384))
    for k in range(128):
        padi[k, k + 128] = 1.0
    got = np.full((NPAIR, 8), np.nan)
    for t in TILES:
        ps = np.zeros((128, 8))
        for s, mb, mlen, rhs, start, stop in t["mms"]:
            ps[mb : mb + mlen] += padi[:, s : s + mlen].T @ srcs[rhs]
        for mb, n, a in t["tts"]:
            ps[mb : mb + n] += srcs[a.split("@")[0]][:n]
        for p in range(128):
            if t["idx"][p] < BIG:
                got[t["idx"][p]] = ps[p]
    ii, jj = np.triu_indices(L)
    exp = Aa[ii] + Cc[jj]
    assert np.allclose(got, exp), "schedule self-check failed"


_schedule_selfcheck()

_CACHE = {}


def _build_nc():
    import concourse.bass as bass
    import concourse.bacc as bacc
    import concourse.mybir as mybir
    import concourse.tile as tile

    f32 = mybir.dt.float32
    bf16 = mybir.dt.bfloat16
    i32 = mybir.dt.int32

    nc = bacc.Bacc(None, target_bir_lowering=False, debug=False)

    seqT = nc.dram_tensor("seqT", [H, L], f32, kind="ExternalInput")
    w1t = nc.dram_tensor("w1t", [H, HH], f32, kind="ExternalInput")
    w2t = nc.dram_tensor("w2t", [H, HH], f32, kind="ExternalInput")
    bias = nc.dram_tensor("bias", [1, HH], f32, kind="ExternalInput")
    ones = nc.dram_tensor("ones", [1, 128], f32, kind="ExternalInput")
    padi = nc.dram_tensor("padi", [128, 384], f32, kind="ExternalInput")
    padib = nc.dram_tensor("padib", [128, 384], bf16, kind="ExternalInput")
    sidx = nc.dram_tensor("sidx", [128, NT], i32, kind="ExternalInput")
    out = nc.dram_tensor("out", [NPAIR, HH], f32, kind="ExternalOutput")

    with tile.TileContext(nc) as tc:
        with (
            tc.tile_pool(name="persist", bufs=1) as pers,
            tc.tile_pool(name="outp", bufs=8) as outp,
        ):
            seqT_sb = [pers.tile([128, L], f32, tag=f"seqT{k}", name=f"seqT{k}") for k in range(6)]
            w1t_sb = [pers.tile([128, HH], f32, tag=f"w1t{k}", name=f"w1t{k}") for k in range(6)]
            w2t_sb = [pers.tile([128, HH], f32, tag=f"w2t{k}", name=f"w2t{k}") for k in range(6)]
            bias_sb = pers.tile([1, HH], f32, tag="bias")
            ones_sb = pers.tile([1, 128], f32, tag="ones")
            padi_sb = pers.tile([128, 384], f32, tag="padi")
            padib_sb = pers.tile([128, 384], bf16, tag="padib")
            sidx_sb = pers.tile([128, NT], i32, tag="sidx")

            for k in range(6):
                nc.sync.dma_start(seqT_sb[k][:], seqT[k * 128 : (k + 1) * 128, :])
                nc.sync.dma_start(w1t_sb[k][:], w1t[k * 128 : (k + 1) * 128, :])
                nc.sync.dma_start(w2t_sb[k][:], w2t[k * 128 : (k + 1) * 128, :])
            nc.sync.dma_start(bias_sb[:], bias[:])
            nc.sync.dma_start(ones_sb[:], ones[:])
            nc.sync.dma_start(padi_sb[:], padi[:])
            nc.sync.dma_start(padib_sb[:], padib[:])
            nc.sync.dma_start(sidx_sb[:], sidx[:])

            # ---- precompute C = seq @ W2^T, A = seq @ W1^T + bias ----
            pre_ctx = tc.tile_pool(name="pre_ps", bufs=2, space="PSUM")
            pre_ps = pre_ctx.__enter__()
            srcs = {}
            for name, wt, add_b, toff in (
                ("C0", w2t_sb, False, 0),
                ("C1", w2t_sb, False, 128),
                ("A0", w1t_sb, True, 0),
                ("A1", w1t_sb, True, 128),
            ):
                ps = pre_ps.tile([128, HH], f32, tag="pre")
                for k in range(6):
                    nc.tensor.matmul(
                        ps[:],
                        lhsT=seqT_sb[k][:, toff : toff + 128],
                        rhs=wt[k][:],
                        start=(k == 0),
                        stop=(k == 5 and not add_b),
                    )
                if add_b:
                    nc.tensor.matmul(
                        ps[:], lhsT=ones_sb[:1, :], rhs=bias_sb[:1, :],
                        start=False, stop=True,
                    )
                dst = pers.tile([128, HH], f32, tag=name, name=name)
                nc.vector.tensor_copy(dst[:], ps[:])
                srcs[name] = dst

            # split each source into bf16 hi + lo so the band matmuls can run
            # as two single-pass bf16 MMs (fp32 PSUM accumulate, ~1e-5 exact)
            hi = {}
            lo = {}
            for name in ("C0", "C1", "A0", "A1"):
                h_t = pers.tile([128, HH], bf16, tag=f"{name}h", name=f"{name}h")
                l_t = pers.tile([128, HH], bf16, tag=f"{name}l", name=f"{name}l")
                nc.vector.tensor_copy(h_t[:], srcs[name][:])
                nc.vector.tensor_sub(l_t[:], srcs[name][:], h_t[:])
                hi[name] = h_t
                lo[name] = l_t

            # shifted copies of A for tail-segment DVE adds:
            # As[f"{name}@{mb}"][mb + m] = A[m]
            for name in ("A0", "A1"):
                for mb in (64, 96):
                    ps = pre_ps.tile([128, HH], f32, tag="pre")
                    nc.tensor.matmul(
                        ps[:],
                        lhsT=padi_sb[:, 128 - mb : 256 - mb],
                        rhs=srcs[name][:],
                        start=True,
                        stop=True,
                    )
                    sh = pers.tile([128, HH], f32, tag=f"{name}s{mb}",
                                   name=f"{name}s{mb}")
                    nc.vector.tensor_copy(sh[:], ps[:])
                    srcs[f"{name}@{mb}"] = sh

            pre_ctx.__exit__(None, None, None)
            mm_ctx = tc.tile_pool(name="mm_ps", bufs=8, space="PSUM")
            mm_ps = mm_ctx.__enter__()

            # ---- main loop over the 288 tiles ----
            for t, tl in enumerate(TILES):
                ps = mm_ps.tile([128, HH], f32, tag="mm")
                nmm = len(tl["mms"])
                for mi, (s, mb, mlen, rhs, start, stop) in enumerate(tl["mms"]):
                    for part, src in ((0, hi[rhs]), (1, lo[rhs])):
                        nc.tensor.matmul(
                            ps[mb : mb + mlen, :],
                            lhsT=padib_sb[:, s : s + mlen],
                            rhs=src[:],
                            start=(mi == 0 and part == 0),
                            stop=(mi == nmm - 1 and part == 1),
                            tile_position=(0, mb),
                        )
                for mb, n, a in tl["tts"]:
                    nc.vector.tensor_add(
                        out=ps[mb : mb + n, :],
                        in0=ps[mb : mb + n, :],
                        in1=srcs[a][mb : mb + n, :],
                    )
                ot = outp.tile([128, HH], f32, tag="ot")
                nc.scalar.activation(
                    ot[:], ps[:], mybir.ActivationFunctionType.Tanh
                )
                nc.gpsimd.indirect_dma_start(
                    out=out[:],
                    out_offset=bass.IndirectOffsetOnAxis(
                        ap=sidx_sb[:, t : t + 1], axis=0
                    ),
                    in_=ot[:],
                    in_offset=None,
                    bounds_check=NPAIR - 1,
                    oob_is_err=False,
                )

            mm_ctx.__exit__(None, None, None)

    nc.compile()
    return nc


def _get_nc():
    if "nc" not in _CACHE:
        _CACHE["nc"] = _build_nc()
    return _CACHE["nc"]


def _host_consts():
    if "consts" in _CACHE:
        return _CACHE["consts"]
    import ml_dtypes

    padi = np.zeros((128, 384), np.float32)
    for k in range(128):
        padi[k, k + 128] = 1.0
    padib = padi.astype(ml_dtypes.bfloat16)
    sidx = np.empty((128, NT), np.int32)
    for t, tl in enumerate(TILES):
        sidx[:, t] = np.minimum(tl["idx"], BIG).astype(np.int32)
    ones = np.ones((1, 128), np.float32)
    _CACHE["consts"] = (padi, padib, sidx, ones)
    return _CACHE["consts"]


def kernel(seq_hiddens, W, b):
    from concourse.bass_utils import run_bass_kernel_spmd

    seq_hiddens = np.asarray(seq_hiddens, dtype=np.float32)
    W = np.asarray(W, dtype=np.float32)
    b = np.asarray(b, dtype=np.float32)

    nc = _get_nc()
    padi, padib, sidx, ones = _host_consts()

    w1T = np.ascontiguousarray(W[:, :H].T)   # [H(k), H(h)]
    w2T = np.ascontiguousarray(W[:, H:].T)

    in_maps = []
    for c in range(8):
        bb, hf = divmod(c, 2)
        hs = slice(hf * HH, (hf + 1) * HH)
        in_maps.append(
            {
                "seqT": np.ascontiguousarray(seq_hiddens[bb].T),
                "w1t": np.ascontiguousarray(w1T[:, hs]),
                "w2t": np.ascontiguousarray(w2T[:, hs]),
                "bias": np.ascontiguousarray(b[hs])[None, :],
                "ones": ones,
                "padi": padi,
                "padib": padib,
                "sidx": sidx,
            }
        )

    res = run_bass_kernel_spmd(nc, in_maps, list(range(8)))
    full = np.empty((B, NPAIR, H), np.float32)
    for bb in range(B):
        full[bb, :, :HH] = res.results[2 * bb]["out"]
        full[bb, :, HH:] = res.results[2 * bb + 1]["out"]
    return full


if __name__ == "__main__":
    rng = np.random.RandomState(0)
    sh = rng.randn(B, L, H).astype(np.float32)
    Wv = (rng.randn(H, 2 * H) * 0.02).astype(np.float32)
    bv = np.zeros(H, np.float32)
    o = kernel(seq_hiddens=sh, W=Wv, b=bv)
    print("kernel output", o.shape, o.dtype, float(np.abs(o).max()))

